# revision 7
# baseline (speedup 1.0000x reference)
"""KANConv2d Trainium2 kernel.

Math: the reference unfolds x into 3x3 patches, applies pointwise nonlinearities
(silu, cos(g*x), sin(g*x), g=1..3) to patch values, and contracts with per-
(in_ch, out_ch, kpos[, g]) weights. Pointwise nonlinearities commute with the
unfold gather, so the whole op is a standard 3x3 conv over a 112-channel
feature map z = concat([cos(g x), sin(g x), silu(x)]) with weights derived
from scale_base / scale_spline*coeff. Padding value of each feature map is
f(0) (cos -> 1, others -> 0).

Sharding: data-parallel over batch, one image per core, parameters replicated.

Row groups (partition-32-aligned for engine ops): rows 0-47 cos g=1..3,
rows 48-95 sin g=1..3, rows 96-111 silu.

Layout per core:
  Xrep  SBUF [112, 9216]  7 copies of x (row group per feature)
  Z     SBUF [112, 9508]  feature maps in padded layout: idx = 1 + R*97 + w,
                          R in [0,98) padded row, w in [0,97) (col 96 = pad);
                          trailing elem so the last shifted matmul read is
                          in-bounds (reads land in excluded pad columns).
  conv: 24 tiles of 4 image rows (N=388 psum cols incl pad cols); per tile
        9 matmuls (one per kernel position) accumulate K=112 x 64 in PSUM;
        rhs = Z[:, (4t+kh)*97 + kw : +388] (contiguous, float32r).
  sin/cos range reduction: v = g*x + phi + 17*pi (always > 0), m = v mod 2pi,
        ACT Sin(m - pi) == sin(g*x + phi). Sin spline valid within ~+-3.55.
"""

import math
import numpy as np

import concourse.bacc as bacc
import concourse.mybir as mybir
from concourse.tile import TileContext
from concourse.bass_utils import run_bass_kernel_spmd

B, C_IN, C_OUT, H, W = 8, 16, 64, 96, 96
G = 3
L = H * W              # 9216
WPAD = W + 1           # 97: one shared zero col between rows
ZLEN = 1 + 98 * WPAD + 1   # leading pad elem + 98 padded rows + trailing elem
ROWS_PER_TILE = 4
NTILES = H // ROWS_PER_TILE      # 24
NCOLS = ROWS_PER_TILE * WPAD     # 388 psum columns per tile
BANDS = 8
BAND_ROWS = H // BANDS           # 12
BAND_ELEMS = BAND_ROWS * W       # 1152
TWO_PI = 2.0 * math.pi
MAGIC = 12582912.0  # 1.5 * 2**23
FP = mybir.dt.float32
FPR = mybir.dt.float32r
AF = mybir.ActivationFunctionType
ALU = mybir.AluOpType

N_CORES = 8

_cache = {}


def _build():
    nc = bacc.Bacc("TRN2", target_bir_lowering=False, debug=False,
                   num_devices=N_CORES)
    x_d = nc.dram_tensor("x", [C_IN, L], FP, kind="ExternalInput")
    w_d = nc.dram_tensor("w", [112, 9 * C_OUT], FPR, kind="ExternalInput")
    s_d = nc.dram_tensor("s", [112, 3], FP, kind="ExternalInput")
    b_d = nc.dram_tensor("b", [C_OUT, 1], FP, kind="ExternalInput")
    p_d = nc.dram_tensor("p", [112, 98], FPR, kind="ExternalInput")
    y_d = nc.dram_tensor("y", [C_OUT, L], FP, kind="ExternalOutput")

    with TileContext(nc) as tc:
        with (
            tc.tile_pool(name="persist", bufs=1) as pp,
            tc.tile_pool(name="out", bufs=6) as op,
            tc.tile_pool(name="rnd", bufs=3) as rp,
            tc.tile_pool(name="psum", bufs=6, space="PSUM") as qp,
        ):
            wt = pp.tile([112, 9 * C_OUT], FPR)
            st = pp.tile([112, 3], FP)
            bt = pp.tile([C_OUT, 1], FP)
            xr = pp.tile([112, L], FP)
            z = pp.tile([112, ZLEN], FPR)

            nc.sync.dma_start(wt[:], w_d.ap())
            nc.sync.dma_start(st[:], s_d.ap())
            nc.sync.dma_start(bt[:], b_d.ap())
            # 7 replicas of x (row groups: 3x cos, 3x sin, silu)
            for grp in range(7):
                nc.sync.dma_start(xr[16 * grp:16 * (grp + 1), :], x_d.ap())

            # Z border fill from per-row pad-value constant (cos rows get 1.0)
            zv = z[:, 1:1 + 98 * WPAD].rearrange("p (r w) -> p r w", r=98, w=WPAD)
            nc.sync.dma_start(z[:, 0:1 + WPAD], p_d.ap())             # lead + top row
            nc.sync.dma_start(z[:, 1 + 97 * WPAD:ZLEN], p_d.ap())     # bottom + tail
            nc.sync.dma_start(zv[:, 1:97, 96:97], p_d.ap()[:, 0:96])  # right pad col

            # features, banded for pipeline overlap with PE; conv tiles
            # are emitted as soon as their input bands are complete so the
            # scheduler starts PE work early
            def conv_tile(t):
                ps = qp.tile([C_OUT, NCOLS], FP)
                idx = 0
                for kh in range(3):
                    for kw in range(3):
                        off = (ROWS_PER_TILE * t + kh) * WPAD + kw
                        nc.tensor.matmul(
                            ps[:, :],
                            wt[:, idx * C_OUT:(idx + 1) * C_OUT],
                            z[:, off:off + NCOLS],
                            start=(idx == 0), stop=(idx == 8),
                        )
                        idx += 1
                ot = op.tile([C_OUT, NCOLS], FP)
                nc.vector.tensor_scalar(ot[:], ps[:], bt[0:C_OUT, 0:1], None,
                                        ALU.add)
                src = ot[:].rearrange("p (r w) -> p r w", r=ROWS_PER_TILE, w=WPAD)
                dst = y_d.ap()[:, t * ROWS_PER_TILE * W:(t + 1) * ROWS_PER_TILE * W]
                dst = dst.rearrange("p (r w) -> p r w", r=ROWS_PER_TILE, w=W)
                nc.sync.dma_start(dst, src[:, :, 0:W])

            emitted = 0
            for bd in range(BANDS):
                sl = slice(bd * BAND_ELEMS, (bd + 1) * BAND_ELEMS)
                vt = xr[0:96, sl]
                nc.vector.tensor_scalar(vt, vt, st[0:96, 0:1], st[0:96, 1:2],
                                        ALU.mult, ALU.add)
                # round-to-int via fp32 magic constant, then frac = u - round(u)
                rt = rp.tile([96, BAND_ELEMS], FP)
                nc.vector.tensor_scalar(rt[:], vt, MAGIC, MAGIC,
                                        ALU.add, ALU.subtract)
                nc.vector.tensor_tensor(vt, vt, rt[:], ALU.subtract)
                src_t = xr[0:96, sl].rearrange("p (r w) -> p r w", r=BAND_ROWS, w=W)
                dst_t = zv[0:96, 1 + bd * BAND_ROWS:1 + (bd + 1) * BAND_ROWS, 0:W]
                nc.scalar.activation(dst_t, src_t, AF.Sin, scale=TWO_PI)
                src_s = xr[96:112, sl].rearrange("p (r w) -> p r w", r=BAND_ROWS, w=W)
                dst_s = zv[96:112, 1 + bd * BAND_ROWS:1 + (bd + 1) * BAND_ROWS, 0:W]
                nc.scalar.activation(dst_s, src_s, AF.Silu)
                # conv tile t reads image rows [4t-1, 4t+5); band bd completes
                # rows < 12*(bd+1)
                while emitted < NTILES and (ROWS_PER_TILE * emitted + 5
                                            <= 12 * (bd + 1) or bd == BANDS - 1):
                    conv_tile(emitted)
                    emitted += 1

    nc.compile()
    return nc


def _prep_params(scale_base, scale_spline, coeff, bias):
    wsb = np.empty((112, 9 * C_OUT), np.float32)
    for g in range(G):
        wc = (scale_spline * coeff[:, 0, :, :, g]).transpose(0, 2, 1)
        ws = (scale_spline * coeff[:, 1, :, :, g]).transpose(0, 2, 1)
        wsb[16 * g:16 * (g + 1)] = wc.reshape(16, 9 * C_OUT)
        wsb[48 + 16 * g:48 + 16 * (g + 1)] = ws.reshape(16, 9 * C_OUT)
    wsb[96:112] = scale_base.transpose(0, 2, 1).reshape(16, 9 * C_OUT)
    s = np.zeros((112, 3), np.float32)
    for g in range(G):
        s[16 * g:16 * (g + 1), 0] = (g + 1.0) / (2 * math.pi)
        s[16 * g:16 * (g + 1), 1] = 0.25 + 8.0
        s[48 + 16 * g:48 + 16 * (g + 1), 0] = (g + 1.0) / (2 * math.pi)
        s[48 + 16 * g:48 + 16 * (g + 1), 1] = 8.0
    pad = np.zeros((112, 98), np.float32)
    pad[0:48] = 1.0
    return wsb, s, np.asarray(bias, np.float32).reshape(C_OUT, 1), pad


def kernel(x, scale_base, scale_spline, coeff, bias, _trace=False):
    x = np.ascontiguousarray(np.asarray(x, np.float32))
    wsb, s, bvec, pad = _prep_params(np.asarray(scale_base, np.float32),
                                     np.asarray(scale_spline, np.float32),
                                     np.asarray(coeff, np.float32),
                                     bias)
    if "nc" not in _cache:
        _cache["nc"] = _build()
    nc = _cache["nc"]
    in_maps = [
        {"x": x[b].reshape(C_IN, L), "w": wsb, "s": s, "b": bvec, "p": pad}
        for b in range(B)
    ]
    res = run_bass_kernel_spmd(nc, in_maps, list(range(N_CORES)), trace=_trace)
    _cache["last_exec_ns"] = res.exec_time_ns
    _cache["last_result"] = res
    out = np.stack([res.results[b]["y"].reshape(C_OUT, H, W) for b in range(B)])
    return out
